# revision 10
# baseline (speedup 1.0000x reference)
"""DeepSATConv v3: T=256 dst tiles + per-region src dedup + host-built S.

Same math as v2 (kernel.py).  Differences:
  - dst tiles of 256 nodes (NT=10): acc spans 2 PSUM banks, 2 selector
    matmuls per chunk (S halves as stationary operands).
  - per (tile, src-block) the edge list is deduplicated by src: one
    gathered slot serves up to 2 edges (distinct dsts) of the same src;
    a src with k edges uses ceil(k/2) slots.  This cuts Q7 descriptor
    generation (the serial bottleneck at ~7ns/slot) by ~10-15%.
  - S is no longer one-hot-by-construction, so it is built on host
    (bf16 counts, mostly 0/1, sometimes 2) and DMA'd per tile.
"""

import numpy as np

N_NODES = 20000
N_EDGES = 320000
D = 256
CORES = 8
NPC = N_NODES // CORES          # 2500 nodes per core
TS = 256                        # dst tile size
NT = (NPC + TS - 1) // TS       # 10 dst tiles per core
NROWS = NT * TS                 # 2560 padded rows per core
NT_ALL = 160                    # phase-A tiles
NPAD = NT_ALL * 128             # 20480
NPB = 2
NT_B0 = 56                      # phase-A tiles in src block 0 (35%)
NBH0 = NT_B0 * 128              # 6144 rows
NBH1 = NPAD - NBH0              # 14336 rows

_cache = {}


def _build(caps):
    import concourse.bacc as bacc
    import concourse.mybir as mybir
    from concourse.tile import TileContext

    nc = bacc.Bacc("TRN2")
    f32 = mybir.dt.float32
    bf16 = mybir.dt.bfloat16

    NCH = sum(sum(r) for r in caps)     # total chunks across tiles/blocks
    NIX = 128 * NCH                     # total gathered slots
    CMAX = [max(c[s] for c in caps) for s in range(NPB)]
    CTMAX = max(c[0] + c[1] for c in caps)

    hT_d = nc.dram_tensor("hT", [128, NT_ALL, 2, 128], bf16, kind="ExternalInput")
    hrow_d = nc.dram_tensor("hrow", [128, NT_ALL, D], bf16, kind="ExternalInput")
    WT_d = nc.dram_tensor("WT", [128, 2, D], bf16, kind="ExternalInput")
    S_d = nc.dram_tensor("S", [128, NCH, TS], bf16, kind="ExternalInput")
    idx_d = nc.dram_tensor("idx", [128, NIX // 16], mybir.dt.int16, kind="ExternalInput")
    out_d = nc.dram_tensor("out", [NROWS, D], f32, kind="ExternalOutput")

    with TileContext(nc) as tc:
        with (
            tc.tile_pool(name="const", bufs=1) as constp,
            tc.tile_pool(name="pha", bufs=3) as pha,
            tc.tile_pool(name="gat0", bufs=NT) as gat0,
            tc.tile_pool(name="gat1", bufs=2) as gat1,
            tc.tile_pool(name="swk", bufs=2) as swk,
            tc.tile_pool(name="fin", bufs=2) as fin,
            tc.tile_pool(name="psa", bufs=2, space="PSUM") as psa,
            tc.tile_pool(name="psb", bufs=2, space="PSUM") as psb,
            tc.tile_pool(name="dram", bufs=1, space="DRAM") as dramp,
        ):
            z_blk = []
            for s_ in range(NPB):
                zb = dramp.tile(
                    [128, (NBH0 if s_ == 0 else NBH1) // 128, 2 * D], bf16,
                    tag=f"zblk{s_}",
                )
                z_blk.append(zb)

            WT_sb = constp.tile([128, 2, D], bf16)
            nc.sync.dma_start(WT_sb[:, :, :], WT_d[:, :, :])
            idx_sb = constp.tile([128, NIX // 16], mybir.dt.int16)
            nc.sync.dma_start(idx_sb[:, :], idx_d[:, :])

            # ---- phase A: Z = [exp(h @ W.T) | exp(h @ W.T) * h], all nodes ----
            # batches of G=4 node-tiles: one DMA / matmul-chain / exp / mult
            # per group to amortize DMA fixed latency and instruction overhead
            G = 2
            for g in range(NT_ALL // G):
                i0 = g * G
                hT_sb = pha.tile([128, G, 2, 128], bf16, tag="hT")
                nc.sync.dma_start(hT_sb[:, :, :, :], hT_d[:, i0:i0 + G, :, :])
                ps = psa.tile([128, G, D], f32, tag="ps")
                for u in range(G):
                    for kb in range(2):
                        nc.tensor.matmul(
                            ps[:, u, :], hT_sb[:, u, kb, :], WT_sb[:, kb, :],
                            start=(kb == 0), stop=(kb == 1),
                        )
                z_sb = pha.tile([128, G, 2 * D], bf16, tag="zs")
                nc.scalar.activation(
                    z_sb[:, :, 0:D], ps[:, :, :], mybir.ActivationFunctionType.Exp
                )
                hr_sb = pha.tile([128, G, D], bf16, tag="hr")
                nc.sync.dma_start(hr_sb[:, :, :], hrow_d[:, i0:i0 + G, :])
                nc.vector.tensor_tensor(
                    z_sb[:, :, D:2 * D], z_sb[:, :, 0:D], hr_sb[:, :, :],
                    mybir.AluOpType.mult,
                )
                if i0 < NT_B0:
                    nc.sync.dma_start(z_blk[0][:, i0:i0 + G, :], z_sb[:, :, :])
                else:
                    li = i0 - NT_B0
                    nc.sync.dma_start(z_blk[1][:, li:li + G, :], z_sb[:, :, :])

            # chunk offsets in host layout: all (t, s=0) regions, then (t, s=1)
            off0 = [0] * NT
            off1 = [0] * NT
            o = 0
            for t in range(NT):
                off0[t] = o
                o += caps[t][0]
            for t in range(NT):
                off1[t] = o
                o += caps[t][1]

            # ---- phase B head: all block-0 gathers (Q7 stream never stalls) ----
            zx0 = []
            for t in range(NT):
                zt = gat0.tile([128, CMAX[0], 2 * D], bf16, tag="zx0")
                Cs = caps[t][0]
                io = off0[t] * 8
                nc.gpsimd.dma_gather(
                    zt[:, 0:Cs, :], z_blk[0][:, :, :].flatten_outer_dims(),
                    idx_sb[:, io:io + 8 * Cs], 128 * Cs, 128 * Cs, 2 * D,
                    single_packet=False,
                )
                zx0.append(zt)

            # ---- phase B: per-tile block-1 gather, selector matmuls, finalize ----
            for t in range(NT):
                C0, C1 = caps[t][0], caps[t][1]
                zx1 = gat1.tile([128, CMAX[1], 2 * D], bf16, tag="zx1")
                io = off1[t] * 8
                nc.gpsimd.dma_gather(
                    zx1[:, 0:C1, :], z_blk[1][:, :, :].flatten_outer_dims(),
                    idx_sb[:, io:io + 8 * C1], 128 * C1, 128 * C1, 2 * D,
                    single_packet=False,
                )
                S_sb = swk.tile([128, CTMAX, TS], bf16, tag="S")
                nc.sync.dma_start(S_sb[:, 0:C0, :], S_d[:, off0[t]:off0[t] + C0, :])
                nc.sync.dma_start(S_sb[:, C0:C0 + C1, :], S_d[:, off1[t]:off1[t] + C1, :])
                acc0 = psb.tile([128, 2 * D], f32, tag="acc0")
                acc1 = psb.tile([128, 2 * D], f32, tag="acc1")
                for j in range(C0 + C1):
                    src_t = zx0[t] if j < C0 else zx1
                    jj = j if j < C0 else j - C0
                    nc.tensor.matmul(
                        acc0[:, :], S_sb[:, j, 0:128], src_t[:, jj, :],
                        start=(j == 0), stop=(j == C0 + C1 - 1),
                    )
                    nc.tensor.matmul(
                        acc1[:, :], S_sb[:, j, 128:256], src_t[:, jj, :],
                        start=(j == 0), stop=(j == C0 + C1 - 1),
                    )

                # ---- finalize tile (two 128-dst halves) ----
                for half, acc in ((0, acc0), (1, acc1)):
                    rec = fin.tile([128, D], f32, tag="rec")
                    nc.vector.reciprocal(rec[:, :], acc[:, 0:D])
                    res = fin.tile([128, D], f32, tag="res")
                    nc.vector.tensor_tensor(
                        res[:, :], acc[:, D:2 * D], rec[:, :],
                        mybir.AluOpType.mult,
                    )
                    ro = t * TS + half * 128
                    nc.sync.dma_start(out_d[ro:ro + 128, :], res[:, :])
    nc.compile()
    return nc


def _wrap_idx(ix):
    w = ix.astype(np.int16).reshape(-1, 8, 16).transpose(2, 0, 1).reshape(16, -1)
    return np.tile(w, (8, 1))


def _dedup_slots(ss, dl):
    """Slots of (src, [dsts]) with <=2 edges per slot, same src per slot.

    Returns (slot_src, slot_of_edge) with edges in (ss, dl) order.
    """
    n = len(ss)
    if n == 0:
        return np.zeros(0, dtype=np.int64), np.zeros(0, dtype=np.int64)
    u, inv = np.unique(ss, return_inverse=True)
    return u, inv


def kernel(h, W_nb, b_nb, W_self, b_self, src, dst):
    from concourse.bass_utils import run_bass_kernel_spmd
    import ml_dtypes

    bf = ml_dtypes.bfloat16
    h = np.ascontiguousarray(np.asarray(h, dtype=np.float32))
    W = np.asarray(W_self, dtype=np.float32)
    src = np.asarray(src, dtype=np.int64)
    dst = np.asarray(dst, dtype=np.int64)

    order = np.argsort(dst, kind="stable")
    src_s = src[order]
    dst_s = dst[order]

    tile_base = []
    for c in range(CORES):
        for t in range(NT):
            tile_base.append(c * NPC + t * TS)
    bounds_lo = np.searchsorted(dst_s, np.array(tile_base), side="left")
    hi_nodes = [min(b + TS, (b // NPC + 1) * NPC) for b in tile_base]
    bounds_hi = np.searchsorted(dst_s, np.array(hi_nodes), side="left")

    # dedup slots per (core, tile, src-block)
    per_ct = {}
    cnt = np.zeros((CORES, NT, NPB), dtype=np.int64)
    for c in range(CORES):
        for t in range(NT):
            i = c * NT + t
            lo, hi = int(bounds_lo[i]), int(bounds_hi[i])
            e_src = src_s[lo:hi]
            e_dst = dst_s[lo:hi] - tile_base[i]
            blk = (e_src >= NBH0).astype(np.int64)
            for s_ in range(NPB):
                sel = np.nonzero(blk == s_)[0]
                slot_src, slot_of_edge = _dedup_slots(e_src[sel], e_dst[sel])
                per_ct[(c, t, s_)] = (slot_src, slot_of_edge, e_dst[sel])
                cnt[c, t, s_] = len(slot_src)
    caps = [
        [int((cnt[:, t, s_].max() + 127) // 128) for s_ in range(NPB)]
        for t in range(NT)
    ]
    NCH = sum(sum(r) for r in caps)

    hp = np.zeros((NPAD, D), dtype=bf)
    hp[:N_NODES] = h.astype(bf)
    hT = np.ascontiguousarray(
        hp.reshape(NT_ALL, 128, 2, 128).transpose(3, 0, 2, 1)
    )
    hrow = np.ascontiguousarray(hp.reshape(NT_ALL, 128, D).transpose(1, 0, 2))
    WT = np.ascontiguousarray(
        W.T.astype(bf).reshape(2, 128, D).transpose(1, 0, 2)
    )

    regions = [(t, 0) for t in range(NT)] + [(t, 1) for t in range(NT)]

    in_maps = []
    for c in range(CORES):
        idx_parts = []
        S_all = np.zeros((128, NCH, TS), dtype=np.float32)
        coff = 0
        for t, s_ in regions:
            Cs = caps[t][s_]
            CAPs = 128 * Cs
            slot_src, slot_of_edge, e_dst = per_ct[(c, t, s_)]
            n = len(slot_src)
            spad = np.zeros(CAPs, dtype=np.int64)
            bl = slot_src - s_ * NBH0      # block-local node index
            K = (NBH0 if s_ == 0 else NBH1) // 128
            spad[:n] = (bl % 128) * K + bl // 128   # p-major row
            idx_parts.append(_wrap_idx(spad))
            np.add.at(
                S_all,
                (slot_of_edge % 128, coff + slot_of_edge // 128, e_dst),
                1.0,
            )
            coff += Cs
        in_maps.append({
            "hT": hT,
            "hrow": hrow,
            "WT": WT,
            "S": S_all.astype(bf),
            "idx": np.ascontiguousarray(np.concatenate(idx_parts, axis=1)),
        })

    key = tuple(tuple(r) for r in caps)
    if key not in _cache:
        _cache[key] = _build(caps)
    nc = _cache[key]

    res = run_bass_kernel_spmd(nc, in_maps, core_ids=list(range(CORES)))
    out = np.concatenate(
        [res.results[c]["out"][:NPC] for c in range(CORES)], axis=0
    ).astype(np.float32)

    deg = np.bincount(dst.astype(np.int64), minlength=N_NODES)
    out[deg == 0] = h[deg == 0]
    return out


# revision 12
# speedup vs baseline: 1.0697x; 1.0697x over previous
"""DeepSATConv v3: T=256 dst tiles + per-region src dedup + host-built S.

Same math as v2 (kernel.py).  Differences:
  - dst tiles of 256 nodes (NT=10): acc spans 2 PSUM banks, 2 selector
    matmuls per chunk (S halves as stationary operands).
  - per (tile, src-block) the edge list is deduplicated by src: one
    gathered slot serves up to 2 edges (distinct dsts) of the same src;
    a src with k edges uses ceil(k/2) slots.  This cuts Q7 descriptor
    generation (the serial bottleneck at ~7ns/slot) by ~10-15%.
  - S is no longer one-hot-by-construction, so it is built on host
    (bf16 counts, mostly 0/1, sometimes 2) and DMA'd per tile.
"""

import numpy as np

N_NODES = 20000
N_EDGES = 320000
D = 256
CORES = 8
NPC = N_NODES // CORES          # 2500 nodes per core
TS = 256                        # dst tile size
NT = (NPC + TS - 1) // TS       # 10 dst tiles per core
NROWS = NT * TS                 # 2560 padded rows per core
NT_ALL = 160                    # phase-A tiles
NPAD = NT_ALL * 128             # 20480
NPB = 2
NT_B0 = 48                      # phase-A tiles in src block 0 (30%)
NBH0 = NT_B0 * 128              # 6144 rows
NBH1 = NPAD - NBH0              # 14336 rows

_cache = {}


def _build(caps):
    import concourse.bacc as bacc
    import concourse.mybir as mybir
    from concourse.tile import TileContext

    nc = bacc.Bacc("TRN2")
    f32 = mybir.dt.float32
    bf16 = mybir.dt.bfloat16

    NCH = sum(sum(r) for r in caps)     # total chunks across tiles/blocks
    NIX = 128 * NCH                     # total gathered slots
    CMAX = [max(c[s] for c in caps) for s in range(NPB)]
    CTMAX = max(c[0] + c[1] for c in caps)

    hT_d = nc.dram_tensor("hT", [128, NT_ALL, 2, 128], bf16, kind="ExternalInput")
    hrow_d = nc.dram_tensor("hrow", [128, NT_ALL, D], bf16, kind="ExternalInput")
    WT_d = nc.dram_tensor("WT", [128, 2, D], bf16, kind="ExternalInput")
    S_d = nc.dram_tensor("S", [128, NCH, TS], bf16, kind="ExternalInput")
    idx_d = nc.dram_tensor("idx", [128, NIX // 16], mybir.dt.int16, kind="ExternalInput")
    out_d = nc.dram_tensor("out", [NROWS, D], f32, kind="ExternalOutput")

    with TileContext(nc) as tc:
        with (
            tc.tile_pool(name="const", bufs=1) as constp,
            tc.tile_pool(name="pha", bufs=3) as pha,
            tc.tile_pool(name="gat0", bufs=NT) as gat0,
            tc.tile_pool(name="gat1", bufs=2) as gat1,
            tc.tile_pool(name="swk", bufs=2) as swk,
            tc.tile_pool(name="fin", bufs=2) as fin,
            tc.tile_pool(name="psa", bufs=2, space="PSUM") as psa,
            tc.tile_pool(name="psb", bufs=2, space="PSUM") as psb,
            tc.tile_pool(name="dram", bufs=1, space="DRAM") as dramp,
        ):
            z_blk = []
            for s_ in range(NPB):
                zb = dramp.tile(
                    [128, (NBH0 if s_ == 0 else NBH1) // 128, 2 * D], bf16,
                    tag=f"zblk{s_}",
                )
                z_blk.append(zb)

            WT_sb = constp.tile([128, 2, D], bf16)
            nc.sync.dma_start(WT_sb[:, :, :], WT_d[:, :, :])
            idx_sb = constp.tile([128, NIX // 16], mybir.dt.int16)
            nc.sync.dma_start(idx_sb[:, :], idx_d[:, :])

            # ---- phase A: Z = [exp(h @ W.T) | exp(h @ W.T) * h], all nodes ----
            # batches of G=4 node-tiles: one DMA / matmul-chain / exp / mult
            # per group to amortize DMA fixed latency and instruction overhead
            G = 4
            for g in range(NT_ALL // G):
                i0 = g * G
                hT_sb = pha.tile([128, G, 2, 128], bf16, tag="hT")
                nc.sync.dma_start(hT_sb[:, :, :, :], hT_d[:, i0:i0 + G, :, :])
                ps = psa.tile([128, G, D], f32, tag="ps")
                for u in range(G):
                    for kb in range(2):
                        nc.tensor.matmul(
                            ps[:, u, :], hT_sb[:, u, kb, :], WT_sb[:, kb, :],
                            start=(kb == 0), stop=(kb == 1),
                        )
                z_sb = pha.tile([128, G, 2 * D], bf16, tag="zs")
                nc.scalar.activation(
                    z_sb[:, :, 0:D], ps[:, :, :], mybir.ActivationFunctionType.Exp
                )
                hr_sb = pha.tile([128, G, D], bf16, tag="hr")
                nc.sync.dma_start(hr_sb[:, :, :], hrow_d[:, i0:i0 + G, :])
                nc.vector.tensor_tensor(
                    z_sb[:, :, D:2 * D], z_sb[:, :, 0:D], hr_sb[:, :, :],
                    mybir.AluOpType.mult,
                )
                if i0 < NT_B0:
                    nc.sync.dma_start(z_blk[0][:, i0:i0 + G, :], z_sb[:, :, :])
                else:
                    li = i0 - NT_B0
                    nc.sync.dma_start(z_blk[1][:, li:li + G, :], z_sb[:, :, :])

            # chunk offsets in host layout: all (t, s=0) regions, then (t, s=1)
            off0 = [0] * NT
            off1 = [0] * NT
            o = 0
            for t in range(NT):
                off0[t] = o
                o += caps[t][0]
            for t in range(NT):
                off1[t] = o
                o += caps[t][1]

            # ---- phase B head: all block-0 gathers (Q7 stream never stalls) ----
            zx0 = []
            for t in range(NT):
                zt = gat0.tile([128, CMAX[0], 2 * D], bf16, tag="zx0")
                Cs = caps[t][0]
                io = off0[t] * 8
                nc.gpsimd.dma_gather(
                    zt[:, 0:Cs, :], z_blk[0][:, :, :].flatten_outer_dims(),
                    idx_sb[:, io:io + 8 * Cs], 128 * Cs, 128 * Cs, 2 * D,
                    single_packet=False,
                )
                zx0.append(zt)

            # ---- phase B: per-tile block-1 gather, selector matmuls, finalize ----
            for t in range(NT):
                C0, C1 = caps[t][0], caps[t][1]
                zx1 = gat1.tile([128, CMAX[1], 2 * D], bf16, tag="zx1")
                io = off1[t] * 8
                nc.gpsimd.dma_gather(
                    zx1[:, 0:C1, :], z_blk[1][:, :, :].flatten_outer_dims(),
                    idx_sb[:, io:io + 8 * C1], 128 * C1, 128 * C1, 2 * D,
                    single_packet=False,
                )
                S_sb = swk.tile([128, CTMAX, TS], bf16, tag="S")
                nc.sync.dma_start(S_sb[:, 0:C0, :], S_d[:, off0[t]:off0[t] + C0, :])
                nc.sync.dma_start(S_sb[:, C0:C0 + C1, :], S_d[:, off1[t]:off1[t] + C1, :])
                acc0 = psb.tile([128, 2 * D], f32, tag="acc0")
                acc1 = psb.tile([128, 2 * D], f32, tag="acc1")
                for j in range(C0 + C1):
                    src_t = zx0[t] if j < C0 else zx1
                    jj = j if j < C0 else j - C0
                    nc.tensor.matmul(
                        acc0[:, :], S_sb[:, j, 0:128], src_t[:, jj, :],
                        start=(j == 0), stop=(j == C0 + C1 - 1),
                    )
                    nc.tensor.matmul(
                        acc1[:, :], S_sb[:, j, 128:256], src_t[:, jj, :],
                        start=(j == 0), stop=(j == C0 + C1 - 1),
                    )

                # ---- finalize tile (two 128-dst halves) ----
                for half, acc in ((0, acc0), (1, acc1)):
                    rec = fin.tile([128, D], f32, tag="rec")
                    nc.vector.reciprocal(rec[:, :], acc[:, 0:D])
                    res = fin.tile([128, D], f32, tag="res")
                    nc.vector.tensor_tensor(
                        res[:, :], acc[:, D:2 * D], rec[:, :],
                        mybir.AluOpType.mult,
                    )
                    ro = t * TS + half * 128
                    nc.sync.dma_start(out_d[ro:ro + 128, :], res[:, :])
    nc.compile()
    return nc


def _wrap_idx(ix):
    w = ix.astype(np.int16).reshape(-1, 8, 16).transpose(2, 0, 1).reshape(16, -1)
    return np.tile(w, (8, 1))


def _dedup_slots(ss, dl):
    """Slots of (src, [dsts]) with <=2 edges per slot, same src per slot.

    Returns (slot_src, slot_of_edge) with edges in (ss, dl) order.
    """
    n = len(ss)
    if n == 0:
        return np.zeros(0, dtype=np.int64), np.zeros(0, dtype=np.int64)
    u, inv = np.unique(ss, return_inverse=True)
    return u, inv


def kernel(h, W_nb, b_nb, W_self, b_self, src, dst):
    from concourse.bass_utils import run_bass_kernel_spmd
    import ml_dtypes

    bf = ml_dtypes.bfloat16
    h = np.ascontiguousarray(np.asarray(h, dtype=np.float32))
    W = np.asarray(W_self, dtype=np.float32)
    src = np.asarray(src, dtype=np.int64)
    dst = np.asarray(dst, dtype=np.int64)

    order = np.argsort(dst, kind="stable")
    src_s = src[order]
    dst_s = dst[order]

    tile_base = []
    for c in range(CORES):
        for t in range(NT):
            tile_base.append(c * NPC + t * TS)
    bounds_lo = np.searchsorted(dst_s, np.array(tile_base), side="left")
    hi_nodes = [min(b + TS, (b // NPC + 1) * NPC) for b in tile_base]
    bounds_hi = np.searchsorted(dst_s, np.array(hi_nodes), side="left")

    # dedup slots per (core, tile, src-block)
    per_ct = {}
    cnt = np.zeros((CORES, NT, NPB), dtype=np.int64)
    for c in range(CORES):
        for t in range(NT):
            i = c * NT + t
            lo, hi = int(bounds_lo[i]), int(bounds_hi[i])
            e_src = src_s[lo:hi]
            e_dst = dst_s[lo:hi] - tile_base[i]
            blk = (e_src >= NBH0).astype(np.int64)
            for s_ in range(NPB):
                sel = np.nonzero(blk == s_)[0]
                slot_src, slot_of_edge = _dedup_slots(e_src[sel], e_dst[sel])
                per_ct[(c, t, s_)] = (slot_src, slot_of_edge, e_dst[sel])
                cnt[c, t, s_] = len(slot_src)
    caps = [
        [int((cnt[:, t, s_].max() + 127) // 128) for s_ in range(NPB)]
        for t in range(NT)
    ]
    NCH = sum(sum(r) for r in caps)

    hp = np.zeros((NPAD, D), dtype=bf)
    hp[:N_NODES] = h.astype(bf)
    hT = np.ascontiguousarray(
        hp.reshape(NT_ALL, 128, 2, 128).transpose(3, 0, 2, 1)
    )
    hrow = np.ascontiguousarray(hp.reshape(NT_ALL, 128, D).transpose(1, 0, 2))
    WT = np.ascontiguousarray(
        W.T.astype(bf).reshape(2, 128, D).transpose(1, 0, 2)
    )

    regions = [(t, 0) for t in range(NT)] + [(t, 1) for t in range(NT)]

    in_maps = []
    for c in range(CORES):
        idx_parts = []
        S_all = np.zeros((128, NCH, TS), dtype=np.float32)
        coff = 0
        for t, s_ in regions:
            Cs = caps[t][s_]
            CAPs = 128 * Cs
            slot_src, slot_of_edge, e_dst = per_ct[(c, t, s_)]
            n = len(slot_src)
            spad = np.zeros(CAPs, dtype=np.int64)
            bl = slot_src - s_ * NBH0      # block-local node index
            K = (NBH0 if s_ == 0 else NBH1) // 128
            spad[:n] = (bl % 128) * K + bl // 128   # p-major row
            idx_parts.append(_wrap_idx(spad))
            np.add.at(
                S_all,
                (slot_of_edge % 128, coff + slot_of_edge // 128, e_dst),
                1.0,
            )
            coff += Cs
        in_maps.append({
            "hT": hT,
            "hrow": hrow,
            "WT": WT,
            "S": S_all.astype(bf),
            "idx": np.ascontiguousarray(np.concatenate(idx_parts, axis=1)),
        })

    key = tuple(tuple(r) for r in caps)
    if key not in _cache:
        _cache[key] = _build(caps)
    nc = _cache[key]

    res = run_bass_kernel_spmd(nc, in_maps, core_ids=list(range(CORES)))
    out = np.concatenate(
        [res.results[c]["out"][:NPC] for c in range(CORES)], axis=0
    ).astype(np.float32)

    deg = np.bincount(dst.astype(np.int64), minlength=N_NODES)
    out[deg == 0] = h[deg == 0]
    return out


# revision 13
# speedup vs baseline: 1.1426x; 1.0681x over previous
"""DeepSATConv v3: T=256 dst tiles + per-region src dedup + host-built S.

Same math as v2 (kernel.py).  Differences:
  - dst tiles of 256 nodes (NT=10): acc spans 2 PSUM banks, 2 selector
    matmuls per chunk (S halves as stationary operands).
  - per (tile, src-block) the edge list is deduplicated by src: one
    gathered slot serves up to 2 edges (distinct dsts) of the same src;
    a src with k edges uses ceil(k/2) slots.  This cuts Q7 descriptor
    generation (the serial bottleneck at ~7ns/slot) by ~10-15%.
  - S is no longer one-hot-by-construction, so it is built on host
    (bf16 counts, mostly 0/1, sometimes 2) and DMA'd per tile.
"""

import numpy as np

N_NODES = 20000
N_EDGES = 320000
D = 256
CORES = 8
NPC = N_NODES // CORES          # 2500 nodes per core
TS = 256                        # dst tile size
NT = (NPC + TS - 1) // TS       # 10 dst tiles per core
NROWS = NT * TS                 # 2560 padded rows per core
NT_ALL = 160                    # phase-A tiles
NPAD = NT_ALL * 128             # 20480
NPB = 2
NT_B0 = 48                      # phase-A tiles in src block 0 (30%)
NBH0 = NT_B0 * 128              # 6144 rows
NBH1 = NPAD - NBH0              # 14336 rows

_cache = {}


def _build(caps, NT_ALL, NT_B0):
    NPAD = NT_ALL * 128
    NBH0 = NT_B0 * 128
    NBH1 = NPAD - NBH0
    import concourse.bacc as bacc
    import concourse.mybir as mybir
    from concourse.tile import TileContext

    nc = bacc.Bacc("TRN2")
    f32 = mybir.dt.float32
    bf16 = mybir.dt.bfloat16

    NCH = sum(sum(r) for r in caps)     # total chunks across tiles/blocks
    NIX = 128 * NCH                     # total gathered slots
    CMAX = [max(c[s] for c in caps) for s in range(NPB)]
    CTMAX = max(c[0] + c[1] for c in caps)

    hT_d = nc.dram_tensor("hT", [128, NT_ALL, 2, 128], bf16, kind="ExternalInput")
    hrow_d = nc.dram_tensor("hrow", [128, NT_ALL, D], bf16, kind="ExternalInput")
    WT_d = nc.dram_tensor("WT", [128, 2, D], bf16, kind="ExternalInput")
    S_d = nc.dram_tensor("S", [128, NCH, TS], bf16, kind="ExternalInput")
    idx_d = nc.dram_tensor("idx", [128, NIX // 16], mybir.dt.int16, kind="ExternalInput")
    out_d = nc.dram_tensor("out", [NROWS, D], f32, kind="ExternalOutput")

    with TileContext(nc) as tc:
        with (
            tc.tile_pool(name="const", bufs=1) as constp,
            tc.tile_pool(name="pha", bufs=3) as pha,
            tc.tile_pool(name="gat0", bufs=NT) as gat0,
            tc.tile_pool(name="gat1", bufs=2) as gat1,
            tc.tile_pool(name="swk", bufs=2) as swk,
            tc.tile_pool(name="fin", bufs=2) as fin,
            tc.tile_pool(name="psa", bufs=2, space="PSUM") as psa,
            tc.tile_pool(name="psb", bufs=2, space="PSUM") as psb,
            tc.tile_pool(name="dram", bufs=1, space="DRAM") as dramp,
        ):
            z_blk = []
            for s_ in range(NPB):
                zb = dramp.tile(
                    [128, (NBH0 if s_ == 0 else NBH1) // 128, 2 * D], bf16,
                    tag=f"zblk{s_}",
                )
                z_blk.append(zb)

            WT_sb = constp.tile([128, 2, D], bf16)
            nc.sync.dma_start(WT_sb[:, :, :], WT_d[:, :, :])
            idx_sb = constp.tile([128, NIX // 16], mybir.dt.int16)
            nc.sync.dma_start(idx_sb[:, :], idx_d[:, :])

            # ---- phase A: Z = [exp(h @ W.T) | exp(h @ W.T) * h], all nodes ----
            # batches of G=4 node-tiles: one DMA / matmul-chain / exp / mult
            # per group to amortize DMA fixed latency and instruction overhead
            G = 4
            for g in range(NT_ALL // G):
                i0 = g * G
                hT_sb = pha.tile([128, G, 2, 128], bf16, tag="hT")
                nc.sync.dma_start(hT_sb[:, :, :, :], hT_d[:, i0:i0 + G, :, :])
                ps = psa.tile([128, G, D], f32, tag="ps")
                for u in range(G):
                    for kb in range(2):
                        nc.tensor.matmul(
                            ps[:, u, :], hT_sb[:, u, kb, :], WT_sb[:, kb, :],
                            start=(kb == 0), stop=(kb == 1),
                        )
                z_sb = pha.tile([128, G, 2 * D], bf16, tag="zs")
                nc.scalar.activation(
                    z_sb[:, :, 0:D], ps[:, :, :], mybir.ActivationFunctionType.Exp
                )
                hr_sb = pha.tile([128, G, D], bf16, tag="hr")
                nc.sync.dma_start(hr_sb[:, :, :], hrow_d[:, i0:i0 + G, :])
                nc.vector.tensor_tensor(
                    z_sb[:, :, D:2 * D], z_sb[:, :, 0:D], hr_sb[:, :, :],
                    mybir.AluOpType.mult,
                )
                if i0 < NT_B0:
                    nc.sync.dma_start(z_blk[0][:, i0:i0 + G, :], z_sb[:, :, :])
                else:
                    li = i0 - NT_B0
                    nc.sync.dma_start(z_blk[1][:, li:li + G, :], z_sb[:, :, :])

            # chunk offsets in host layout: all (t, s=0) regions, then (t, s=1)
            off0 = [0] * NT
            off1 = [0] * NT
            o = 0
            for t in range(NT):
                off0[t] = o
                o += caps[t][0]
            for t in range(NT):
                off1[t] = o
                o += caps[t][1]

            # ---- phase B head: all block-0 gathers (Q7 stream never stalls) ----
            zx0 = []
            for t in range(NT):
                zt = gat0.tile([128, CMAX[0], 2 * D], bf16, tag="zx0")
                Cs = caps[t][0]
                io = off0[t] * 8
                nc.gpsimd.dma_gather(
                    zt[:, 0:Cs, :], z_blk[0][:, :, :].flatten_outer_dims(),
                    idx_sb[:, io:io + 8 * Cs], 128 * Cs, 128 * Cs, 2 * D,
                    single_packet=False,
                )
                zx0.append(zt)

            # ---- phase B: per-tile block-1 gather, selector matmuls, finalize ----
            for t in range(NT):
                C0, C1 = caps[t][0], caps[t][1]
                zx1 = gat1.tile([128, CMAX[1], 2 * D], bf16, tag="zx1")
                io = off1[t] * 8
                nc.gpsimd.dma_gather(
                    zx1[:, 0:C1, :], z_blk[1][:, :, :].flatten_outer_dims(),
                    idx_sb[:, io:io + 8 * C1], 128 * C1, 128 * C1, 2 * D,
                    single_packet=False,
                )
                S_sb = swk.tile([128, CTMAX, TS], bf16, tag="S")
                nc.sync.dma_start(S_sb[:, 0:C0, :], S_d[:, off0[t]:off0[t] + C0, :])
                nc.sync.dma_start(S_sb[:, C0:C0 + C1, :], S_d[:, off1[t]:off1[t] + C1, :])
                acc0 = psb.tile([128, 2 * D], f32, tag="acc0")
                acc1 = psb.tile([128, 2 * D], f32, tag="acc1")
                for j in range(C0 + C1):
                    src_t = zx0[t] if j < C0 else zx1
                    jj = j if j < C0 else j - C0
                    nc.tensor.matmul(
                        acc0[:, :], S_sb[:, j, 0:128], src_t[:, jj, :],
                        start=(j == 0), stop=(j == C0 + C1 - 1),
                    )
                    nc.tensor.matmul(
                        acc1[:, :], S_sb[:, j, 128:256], src_t[:, jj, :],
                        start=(j == 0), stop=(j == C0 + C1 - 1),
                    )

                # ---- finalize tile (two 128-dst halves) ----
                for half, acc in ((0, acc0), (1, acc1)):
                    rec = fin.tile([128, D], f32, tag="rec")
                    nc.vector.reciprocal(rec[:, :], acc[:, 0:D])
                    res = fin.tile([128, D], f32, tag="res")
                    nc.vector.tensor_tensor(
                        res[:, :], acc[:, D:2 * D], rec[:, :],
                        mybir.AluOpType.mult,
                    )
                    ro = t * TS + half * 128
                    nc.sync.dma_start(out_d[ro:ro + 128, :], res[:, :])
    nc.compile()
    return nc


def _wrap_idx(ix):
    w = ix.astype(np.int16).reshape(-1, 8, 16).transpose(2, 0, 1).reshape(16, -1)
    return np.tile(w, (8, 1))


def _dedup_slots(ss, dl):
    """Slots of (src, [dsts]) with <=2 edges per slot, same src per slot.

    Returns (slot_src, slot_of_edge) with edges in (ss, dl) order.
    """
    n = len(ss)
    if n == 0:
        return np.zeros(0, dtype=np.int64), np.zeros(0, dtype=np.int64)
    u, inv = np.unique(ss, return_inverse=True)
    return u, inv


def kernel(h, W_nb, b_nb, W_self, b_self, src, dst):
    from concourse.bass_utils import run_bass_kernel_spmd
    import ml_dtypes

    bf = ml_dtypes.bfloat16
    h = np.ascontiguousarray(np.asarray(h, dtype=np.float32))
    W = np.asarray(W_self, dtype=np.float32)
    src = np.asarray(src, dtype=np.int64)
    dst = np.asarray(dst, dtype=np.int64)

    order = np.argsort(dst, kind="stable")
    src_s = src[order]
    dst_s = dst[order]

    tile_base = []
    for c in range(CORES):
        for t in range(NT):
            tile_base.append(c * NPC + t * TS)
    bounds_lo = np.searchsorted(dst_s, np.array(tile_base), side="left")
    hi_nodes = [min(b + TS, (b // NPC + 1) * NPC) for b in tile_base]
    bounds_hi = np.searchsorted(dst_s, np.array(hi_nodes), side="left")

    # per-core renumbering: core c only materializes Z rows it gathers
    used = []
    for c in range(CORES):
        lo = int(bounds_lo[c * NT])
        hi = int(bounds_hi[(c + 1) * NT - 1])
        used.append(np.unique(src_s[lo:hi]))
    NT_ALL_n = (max(len(u) for u in used) + 511) // 512 * 4   # tiles, mult of 4
    NPAD_n = NT_ALL_n * 128
    NT_B0_n = int(round(0.3 * NT_ALL_n / 4)) * 4
    NBH0_n = NT_B0_n * 128

    # dedup slots per (core, tile, src-block) in renumbered space
    per_ct = {}
    cnt = np.zeros((CORES, NT, NPB), dtype=np.int64)
    for c in range(CORES):
        for t in range(NT):
            i = c * NT + t
            lo, hi = int(bounds_lo[i]), int(bounds_hi[i])
            e_src = np.searchsorted(used[c], src_s[lo:hi])   # renumbered id
            e_dst = dst_s[lo:hi] - tile_base[i]
            blk = (e_src >= NBH0_n).astype(np.int64)
            for s_ in range(NPB):
                sel = np.nonzero(blk == s_)[0]
                slot_src, slot_of_edge = _dedup_slots(e_src[sel], e_dst[sel])
                per_ct[(c, t, s_)] = (slot_src, slot_of_edge, e_dst[sel])
                cnt[c, t, s_] = len(slot_src)
    caps = [
        [int((cnt[:, t, s_].max() + 127) // 128) for s_ in range(NPB)]
        for t in range(NT)
    ]
    NCH = sum(sum(r) for r in caps)

    hb = h.astype(bf)
    hTs, hrows = [], []
    for c in range(CORES):
        hp = np.zeros((NPAD_n, D), dtype=bf)
        hp[:len(used[c])] = hb[used[c]]
        hTs.append(np.ascontiguousarray(
            hp.reshape(NT_ALL_n, 128, 2, 128).transpose(3, 0, 2, 1)
        ))
        hrows.append(np.ascontiguousarray(
            hp.reshape(NT_ALL_n, 128, D).transpose(1, 0, 2)
        ))
    WT = np.ascontiguousarray(
        W.T.astype(bf).reshape(2, 128, D).transpose(1, 0, 2)
    )

    regions = [(t, 0) for t in range(NT)] + [(t, 1) for t in range(NT)]

    in_maps = []
    for c in range(CORES):
        idx_parts = []
        S_all = np.zeros((128, NCH, TS), dtype=np.float32)
        coff = 0
        for t, s_ in regions:
            Cs = caps[t][s_]
            CAPs = 128 * Cs
            slot_src, slot_of_edge, e_dst = per_ct[(c, t, s_)]
            n = len(slot_src)
            spad = np.zeros(CAPs, dtype=np.int64)
            bl = slot_src - s_ * NBH0_n    # block-local renumbered index
            K = (NBH0_n if s_ == 0 else NPAD_n - NBH0_n) // 128
            spad[:n] = (bl % 128) * K + bl // 128   # p-major row
            idx_parts.append(_wrap_idx(spad))
            np.add.at(
                S_all,
                (slot_of_edge % 128, coff + slot_of_edge // 128, e_dst),
                1.0,
            )
            coff += Cs
        in_maps.append({
            "hT": hTs[c],
            "hrow": hrows[c],
            "WT": WT,
            "S": S_all.astype(bf),
            "idx": np.ascontiguousarray(np.concatenate(idx_parts, axis=1)),
        })

    key = (tuple(tuple(r) for r in caps), NT_ALL_n, NT_B0_n)
    if key not in _cache:
        _cache[key] = _build(caps, NT_ALL_n, NT_B0_n)
    nc = _cache[key]

    res = run_bass_kernel_spmd(nc, in_maps, core_ids=list(range(CORES)))
    out = np.concatenate(
        [res.results[c]["out"][:NPC] for c in range(CORES)], axis=0
    ).astype(np.float32)

    deg = np.bincount(dst.astype(np.int64), minlength=N_NODES)
    out[deg == 0] = h[deg == 0]
    return out


# revision 14
# speedup vs baseline: 1.2027x; 1.0526x over previous
"""DeepSATConv v3: T=256 dst tiles + per-region src dedup + host-built S.

Same math as v2 (kernel.py).  Differences:
  - dst tiles of 256 nodes (NT=10): acc spans 2 PSUM banks, 2 selector
    matmuls per chunk (S halves as stationary operands).
  - per (tile, src-block) the edge list is deduplicated by src: one
    gathered slot serves up to 2 edges (distinct dsts) of the same src;
    a src with k edges uses ceil(k/2) slots.  This cuts Q7 descriptor
    generation (the serial bottleneck at ~7ns/slot) by ~10-15%.
  - S is no longer one-hot-by-construction, so it is built on host
    (bf16 counts, mostly 0/1, sometimes 2) and DMA'd per tile.
"""

import numpy as np

N_NODES = 20000
N_EDGES = 320000
D = 256
CORES = 8
NPC = N_NODES // CORES          # 2500 nodes per core
TS = 256                        # dst tile size
NT = (NPC + TS - 1) // TS       # 10 dst tiles per core
NROWS = NT * TS                 # 2560 padded rows per core
NT_ALL = 160                    # phase-A tiles
NPAD = NT_ALL * 128             # 20480
NPB = 2
NT_B0 = 48                      # phase-A tiles in src block 0 (30%)
NBH0 = NT_B0 * 128              # 6144 rows
NBH1 = NPAD - NBH0              # 14336 rows

_cache = {}


def _build(caps, NT_ALL, NT_B0):
    NPAD = NT_ALL * 128
    NBH0 = NT_B0 * 128
    NBH1 = NPAD - NBH0
    import concourse.bacc as bacc
    import concourse.mybir as mybir
    from concourse.tile import TileContext

    nc = bacc.Bacc("TRN2")
    f32 = mybir.dt.float32
    bf16 = mybir.dt.bfloat16

    NCH = sum(sum(r) for r in caps)     # total chunks across tiles/blocks
    NIX = 128 * NCH                     # total gathered slots
    CMAX = [max(c[s] for c in caps) for s in range(NPB)]
    CTMAX = max(c[0] + c[1] for c in caps)

    hT_d = nc.dram_tensor("hT", [128, NT_ALL, 2, 128], bf16, kind="ExternalInput")
    hrow_d = nc.dram_tensor("hrow", [128, NT_ALL, D], bf16, kind="ExternalInput")
    WT_d = nc.dram_tensor("WT", [128, 2, D], bf16, kind="ExternalInput")
    S_d = nc.dram_tensor("S", [128, NCH, TS], bf16, kind="ExternalInput")
    idx_d = nc.dram_tensor("idx", [128, NIX // 16], mybir.dt.int16, kind="ExternalInput")
    out_d = nc.dram_tensor("out", [NROWS, D], f32, kind="ExternalOutput")

    with TileContext(nc) as tc:
        with (
            tc.tile_pool(name="const", bufs=1) as constp,
            tc.tile_pool(name="pha", bufs=3) as pha,
            tc.tile_pool(name="gat0", bufs=NT) as gat0,
            tc.tile_pool(name="gat1", bufs=2) as gat1,
            tc.tile_pool(name="swk", bufs=2) as swk,
            tc.tile_pool(name="fin", bufs=2) as fin,
            tc.tile_pool(name="psa", bufs=2, space="PSUM") as psa,
            tc.tile_pool(name="psb", bufs=2, space="PSUM") as psb,
            tc.tile_pool(name="dram", bufs=1, space="DRAM") as dramp,
        ):
            z_blk = []
            for s_ in range(NPB):
                zb = dramp.tile(
                    [128, (NBH0 if s_ == 0 else NBH1) // 128, 2 * D], bf16,
                    tag=f"zblk{s_}",
                )
                z_blk.append(zb)

            WT_sb = constp.tile([128, 2, D], bf16)
            nc.sync.dma_start(WT_sb[:, :, :], WT_d[:, :, :])
            idx_sb = constp.tile([128, NIX // 16], mybir.dt.int16)
            nc.sync.dma_start(idx_sb[:, :], idx_d[:, :])

            # ---- phase A: Z = [exp(h @ W.T) | exp(h @ W.T) * h], all nodes ----
            # batches of G=4 node-tiles: one DMA / matmul-chain / exp / mult
            # per group to amortize DMA fixed latency and instruction overhead
            G = 4
            for g in range(NT_ALL // G):
                i0 = g * G
                hT_sb = pha.tile([128, G, 2, 128], bf16, tag="hT")
                nc.sync.dma_start(hT_sb[:, :, :, :], hT_d[:, i0:i0 + G, :, :])
                ps = psa.tile([128, G, D], f32, tag="ps")
                for u in range(G):
                    for kb in range(2):
                        nc.tensor.matmul(
                            ps[:, u, :], hT_sb[:, u, kb, :], WT_sb[:, kb, :],
                            start=(kb == 0), stop=(kb == 1),
                        )
                z_sb = pha.tile([128, G, 2 * D], bf16, tag="zs")
                nc.scalar.activation(
                    z_sb[:, :, 0:D], ps[:, :, :], mybir.ActivationFunctionType.Exp
                )
                hr_sb = pha.tile([128, G, D], bf16, tag="hr")
                nc.sync.dma_start(hr_sb[:, :, :], hrow_d[:, i0:i0 + G, :])
                nc.vector.tensor_tensor(
                    z_sb[:, :, D:2 * D], z_sb[:, :, 0:D], hr_sb[:, :, :],
                    mybir.AluOpType.mult,
                )
                if i0 < NT_B0:
                    nc.sync.dma_start(z_blk[0][:, i0:i0 + G, :], z_sb[:, :, :])
                else:
                    li = i0 - NT_B0
                    nc.sync.dma_start(z_blk[1][:, li:li + G, :], z_sb[:, :, :])

            # chunk offsets in host layout: all (t, s=0) regions, then (t, s=1)
            off0 = [0] * NT
            off1 = [0] * NT
            o = 0
            for t in range(NT):
                off0[t] = o
                o += caps[t][0]
            for t in range(NT):
                off1[t] = o
                o += caps[t][1]

            # ---- phase B head: all block-0 gathers (Q7 stream never stalls) ----
            zx0 = []
            for t in range(NT):
                zt = gat0.tile([128, CMAX[0], 2 * D], bf16, tag="zx0")
                Cs = caps[t][0]
                io = off0[t] * 8
                nc.gpsimd.dma_gather(
                    zt[:, 0:Cs, :], z_blk[0][:, :, :].flatten_outer_dims(),
                    idx_sb[:, io:io + 8 * Cs], 128 * Cs, 128 * Cs, 2 * D,
                    single_packet=False,
                )
                zx0.append(zt)

            # ---- phase B: per-tile block-1 gather, selector matmuls, finalize ----
            for t in range(NT):
                C0, C1 = caps[t][0], caps[t][1]
                zx1 = gat1.tile([128, CMAX[1], 2 * D], bf16, tag="zx1")
                io = off1[t] * 8
                nc.gpsimd.dma_gather(
                    zx1[:, 0:C1, :], z_blk[1][:, :, :].flatten_outer_dims(),
                    idx_sb[:, io:io + 8 * C1], 128 * C1, 128 * C1, 2 * D,
                    single_packet=False,
                )
                S_sb = swk.tile([128, CTMAX, TS], bf16, tag="S")
                nc.sync.dma_start(S_sb[:, 0:C0, :], S_d[:, off0[t]:off0[t] + C0, :])
                nc.sync.dma_start(S_sb[:, C0:C0 + C1, :], S_d[:, off1[t]:off1[t] + C1, :])
                acc0 = psb.tile([128, 2 * D], f32, tag="acc0")
                acc1 = psb.tile([128, 2 * D], f32, tag="acc1")
                for j in range(C0 + C1):
                    src_t = zx0[t] if j < C0 else zx1
                    jj = j if j < C0 else j - C0
                    nc.tensor.matmul(
                        acc0[:, :], S_sb[:, j, 0:128], src_t[:, jj, :],
                        start=(j == 0), stop=(j == C0 + C1 - 1),
                    )
                    nc.tensor.matmul(
                        acc1[:, :], S_sb[:, j, 128:256], src_t[:, jj, :],
                        start=(j == 0), stop=(j == C0 + C1 - 1),
                    )

                # ---- finalize tile (two 128-dst halves) ----
                for half, acc in ((0, acc0), (1, acc1)):
                    rec = fin.tile([128, D], f32, tag="rec")
                    nc.vector.reciprocal(rec[:, :], acc[:, 0:D])
                    res = fin.tile([128, D], f32, tag="res")
                    nc.vector.tensor_tensor(
                        res[:, :], acc[:, D:2 * D], rec[:, :],
                        mybir.AluOpType.mult,
                    )
                    ro = t * TS + half * 128
                    nc.sync.dma_start(out_d[ro:ro + 128, :], res[:, :])
    nc.compile()
    return nc


def _wrap_idx(ix):
    w = ix.astype(np.int16).reshape(-1, 8, 16).transpose(2, 0, 1).reshape(16, -1)
    return np.tile(w, (8, 1))


def _dedup_slots(ss, dl):
    """Slots of (src, [dsts]) with <=2 edges per slot, same src per slot.

    Returns (slot_src, slot_of_edge) with edges in (ss, dl) order.
    """
    n = len(ss)
    if n == 0:
        return np.zeros(0, dtype=np.int64), np.zeros(0, dtype=np.int64)
    u, inv = np.unique(ss, return_inverse=True)
    return u, inv


def kernel(h, W_nb, b_nb, W_self, b_self, src, dst):
    from concourse.bass_utils import run_bass_kernel_spmd
    import ml_dtypes

    bf = ml_dtypes.bfloat16
    h = np.ascontiguousarray(np.asarray(h, dtype=np.float32))
    W = np.asarray(W_self, dtype=np.float32)
    src = np.asarray(src, dtype=np.int64)
    dst = np.asarray(dst, dtype=np.int64)

    order = np.argsort(dst, kind="stable")
    src_s = src[order]
    dst_s = dst[order]

    tile_base = []
    for c in range(CORES):
        for t in range(NT):
            tile_base.append(c * NPC + t * TS)
    bounds_lo = np.searchsorted(dst_s, np.array(tile_base), side="left")
    hi_nodes = [min(b + TS, (b // NPC + 1) * NPC) for b in tile_base]
    bounds_hi = np.searchsorted(dst_s, np.array(hi_nodes), side="left")

    # per-core renumbering: core c only materializes Z rows it gathers
    used = []
    for c in range(CORES):
        lo = int(bounds_lo[c * NT])
        hi = int(bounds_hi[(c + 1) * NT - 1])
        used.append(np.unique(src_s[lo:hi]))
    NT_ALL_n = (max(len(u) for u in used) + 511) // 512 * 4   # tiles, mult of 4
    NPAD_n = NT_ALL_n * 128
    NT_B0_n = int(round(0.315 * NT_ALL_n / 4)) * 4
    NBH0_n = NT_B0_n * 128

    # dedup slots per (core, tile, src-block) in renumbered space
    per_ct = {}
    cnt = np.zeros((CORES, NT, NPB), dtype=np.int64)
    for c in range(CORES):
        for t in range(NT):
            i = c * NT + t
            lo, hi = int(bounds_lo[i]), int(bounds_hi[i])
            e_src = np.searchsorted(used[c], src_s[lo:hi])   # renumbered id
            e_dst = dst_s[lo:hi] - tile_base[i]
            blk = (e_src >= NBH0_n).astype(np.int64)
            for s_ in range(NPB):
                sel = np.nonzero(blk == s_)[0]
                slot_src, slot_of_edge = _dedup_slots(e_src[sel], e_dst[sel])
                per_ct[(c, t, s_)] = (slot_src, slot_of_edge, e_dst[sel])
                cnt[c, t, s_] = len(slot_src)
    caps = [
        [int((cnt[:, t, s_].max() + 127) // 128) for s_ in range(NPB)]
        for t in range(NT)
    ]
    NCH = sum(sum(r) for r in caps)

    hb = h.astype(bf)
    hTs, hrows = [], []
    for c in range(CORES):
        hp = np.zeros((NPAD_n, D), dtype=bf)
        hp[:len(used[c])] = hb[used[c]]
        hTs.append(np.ascontiguousarray(
            hp.reshape(NT_ALL_n, 128, 2, 128).transpose(3, 0, 2, 1)
        ))
        hrows.append(np.ascontiguousarray(
            hp.reshape(NT_ALL_n, 128, D).transpose(1, 0, 2)
        ))
    WT = np.ascontiguousarray(
        W.T.astype(bf).reshape(2, 128, D).transpose(1, 0, 2)
    )

    regions = [(t, 0) for t in range(NT)] + [(t, 1) for t in range(NT)]

    in_maps = []
    for c in range(CORES):
        idx_parts = []
        S_all = np.zeros((128, NCH, TS), dtype=np.float32)
        coff = 0
        for t, s_ in regions:
            Cs = caps[t][s_]
            CAPs = 128 * Cs
            slot_src, slot_of_edge, e_dst = per_ct[(c, t, s_)]
            n = len(slot_src)
            spad = np.zeros(CAPs, dtype=np.int64)
            bl = slot_src - s_ * NBH0_n    # block-local renumbered index
            K = (NBH0_n if s_ == 0 else NPAD_n - NBH0_n) // 128
            spad[:n] = (bl % 128) * K + bl // 128   # p-major row
            idx_parts.append(_wrap_idx(spad))
            np.add.at(
                S_all,
                (slot_of_edge % 128, coff + slot_of_edge // 128, e_dst),
                1.0,
            )
            coff += Cs
        in_maps.append({
            "hT": hTs[c],
            "hrow": hrows[c],
            "WT": WT,
            "S": S_all.astype(bf),
            "idx": np.ascontiguousarray(np.concatenate(idx_parts, axis=1)),
        })

    key = (tuple(tuple(r) for r in caps), NT_ALL_n, NT_B0_n)
    if key not in _cache:
        _cache[key] = _build(caps, NT_ALL_n, NT_B0_n)
    nc = _cache[key]

    res = run_bass_kernel_spmd(nc, in_maps, core_ids=list(range(CORES)))
    out = np.concatenate(
        [res.results[c]["out"][:NPC] for c in range(CORES)], axis=0
    ).astype(np.float32)

    deg = np.bincount(dst.astype(np.int64), minlength=N_NODES)
    out[deg == 0] = h[deg == 0]
    return out
